# revision 39
# baseline (speedup 1.0000x reference)
"""Trainium2 Bass kernel for AsymmetricPositionAttentionModule.

Strategy: pure data parallelism — batch B=8 split across 8 NeuronCores, one
image per core. Per-core graph (convs fp8 DoubleRow, attention bf16, fp32 acc):

  qk  = relu(Wqk8'·x8 + bqk)     fp8 DR matmuls -> ACT relu drain (bf16 pin)
  val = Wv8'·x8                  fp8 DR matmuls -> pooled DIRECTLY from PSUM
                                 (relu+bias applied post-pooling on psp: max
                                  commutes with the monotone relu(x+b))
  key/valPSP = PSP maxpool       DVE/Pool max-tree, symmetric-window batching
  scoresT = keyT·qk              PE, [110, 512] per chunk
  esc = exp(scores/16)           ACT, bf16
  sums = onesT·esc               PE broadcast trick
  escn = esc * recip(sums)       DVE recip, ACT cast, DVE mult
  W2T = valPSP·Wout'T            PE [110, 512] (folds out-conv with value)
  z = W2T.T·escn + I·xb          PE, residual via identity matmul (bf16 x)
  out = z + bout                 ACT/DVE epilogue, bf16 out
"""

import sys

sys.path.insert(0, "/opt/trn_rl_repo")

from contextlib import ExitStack

import numpy as np
import ml_dtypes

CIN = 512
CK = 256
CV = 512
NPIX = 4096
S = 110
NT = 8          # pixel columns of 512
COL = 512
EPS = 1e-5
WSCALE = 1024.0   # pow2 boost for folded conv weights into fp8 range
WUNSCALE = 1.0 / WSCALE

_CACHE = {}


def _build():
    import concourse.bass as bass
    import concourse.tile as tile
    from concourse import bacc, mybir

    f32 = mybir.dt.float32
    bf16 = mybir.dt.bfloat16
    fp8 = mybir.dt.float8e4
    ts = bass.ts
    AF = mybir.ActivationFunctionType
    ALU = mybir.AluOpType
    AX = mybir.AxisListType
    PM = mybir.MatmulPerfMode.DoubleRow

    nc = bacc.Bacc("TRN2", target_bir_lowering=False, debug=False, num_devices=8)

    x_d = nc.dram_tensor("x", [4, 128, NPIX], bf16, kind="ExternalInput").ap()
    x8_d = nc.dram_tensor("x8", [4, 128, NPIX], fp8, kind="ExternalInput").ap()
    qkw_d = nc.dram_tensor("qk_wt", [4, 128, CK], fp8, kind="ExternalInput").ap()
    vw_d = nc.dram_tensor("v_wt", [4, 128, CV], fp8, kind="ExternalInput").ap()
    w2_d = nc.dram_tensor("w2_rhs", [4, 128, CIN], bf16, kind="ExternalInput").ap()
    bqk_d = nc.dram_tensor("b_qk", [128, 2], f32, kind="ExternalInput").ap()
    bvx_d = nc.dram_tensor("b_vx", [128, 12], f32, kind="ExternalInput").ap()
    id_d = nc.dram_tensor("ident", [128, 128], bf16, kind="ExternalInput").ap()
    ones_d = nc.dram_tensor("ones", [S, 128], bf16, kind="ExternalInput").ap()
    out_d = nc.dram_tensor("out", [4, 128, NPIX], bf16, kind="ExternalOutput").ap()

    with tile.TileContext(nc) as tc, ExitStack() as ctx:
        const = ctx.enter_context(tc.tile_pool(name="const", bufs=1))
        persist = ctx.enter_context(tc.tile_pool(name="persist", bufs=1))
        rpool = ctx.enter_context(tc.tile_pool(name="rpool", bufs=2))
        opool = ctx.enter_context(tc.tile_pool(name="opool", bufs=2))
        psum = ctx.enter_context(tc.tile_pool(name="psum", bufs=2, space="PSUM"))

        # ---- constants ----
        wqk = const.tile([128, 4, CK], fp8)
        wv = const.tile([128, 4, CV], fp8)
        w2r = const.tile([128, 4, CIN], bf16)
        bqk = const.tile([128, 2], f32)
        bvx = const.tile([128, 12], f32)   # bv 0:4 | -bv 4:8 | bout 8:12
        ident = const.tile([128, 128], bf16)
        ones = const.tile([S, 128], bf16)
        # ---- persistent activations ----
        x32 = persist.tile([128, 4, NPIX], bf16)      # input (bf16), residual only
        x8s = persist.tile([128, 4, NPIX], fp8)       # input (fp8), conv operand
        pin = persist.tile([128, 2, NPIX], fp8)       # qk activations (fp8)
        pspq = persist.tile([128, 2, 128], fp8)       # pooled key, fp8, padded
        rawq = persist.tile([128, 2, 2, 2, 64], bf16) # qk raw rows (d03|d25)
        vin = persist.tile([128, 4, NPIX], bf16)      # val activations
        H2 = persist.tile([128, 6, 32, 64], bf16)
        H4 = persist.tile([128, 6, 16, 64], bf16)
        H8 = persist.tile([128, 6, 8, 64], bf16)
        H6 = persist.tile([128, 6, 6, 64], bf16)
        t36 = persist.tile([128, 6, 3, 6], bf16)
        W1 = persist.tile([128, 6, 8, 8, 4], bf16)
        W2s = persist.tile([128, 6, 8, 8, 2], bf16)
        psp = persist.tile([128, 6, S], bf16)         # pooled: [s1|s3|s6|s8]
        esc = persist.tile([S, NPIX], bf16)
        w2t = persist.tile([S, CIN], bf16)

        # x8 lands group-by-group across 3 queues; a single dma_start only
        # sustains ~40GB/s, so split into per-(k, column-group) pieces.
        Q3 = [nc.sync, nc.scalar, nc.gpsimd]
        for k in range(4):          # group 0 in half-pieces, round-robin
            for h, (a, b) in enumerate([(0, 768), (768, 1536)]):
                Q3[(2 * k + h) % 3].dma_start(x8s[:, k, a:b], x8_d[k][:, a:b])
        for k in range(4):
            nc.scalar.dma_start(wqk[:, k, :], qkw_d[k])
        nc.scalar.dma_start(bqk[:], bqk_d)
        for gi, (a, b) in enumerate([(1536, 3072), (3072, 4096)]):
            nc.sync.dma_start(x8s[:, 0, a:b], x8_d[0][:, a:b])
            nc.scalar.dma_start(x8s[:, 1, a:b], x8_d[1][:, a:b])
            nc.sync.dma_start(x8s[:, 2, a:b], x8_d[2][:, a:b])
            nc.gpsimd.dma_start(x8s[:, 3, a:b], x8_d[3][:, a:b])
        for k in range(4):
            nc.gpsimd.dma_start(wv[:, k, :], vw_d[k])
            nc.gpsimd.dma_start(w2r[:, k, :], w2_d[k])
        nc.gpsimd.dma_start(bvx[:], bvx_d)
        nc.gpsimd.dma_start(ident[:], id_d)
        nc.gpsimd.dma_start(ones[:], ones_d)
        for k in range(4):
            nc.gpsimd.dma_start(x32[:, k, :], x_d[k])

        nc.vector.memset(pspq[:, :, S:128], 0)

        # views of pin for the pooling tree (free dim = h*64 + w)
        pin_e = pin.rearrange("p b (hp e w) -> p b hp e w", e=2, w=64)
        pin_r = pin.rearrange("p b (e h w) -> p b e h w", e=2, w=64)  # h half-split
        vin_e = vin.rearrange("p b (hp e w) -> p b hp e w", e=2, w=64)
        vin_r = vin.rearrange("p b (e h w) -> p b e h w", e=2, w=64)
        h2_e = H2.rearrange("p b (hp e) w -> p b hp e w", e=2)
        h4_e = H4.rearrange("p b (hp e) w -> p b hp e w", e=2)
        H2v = H2.rearrange("p b (e h) w -> p b e h w", e=2)
        H4v = H4.rearrange("p b (e h) w -> p b e h w", e=2)
        H8v = H8.rearrange("p b (e h) w -> p b e h w", e=2)
        H6v = H6.rearrange("p b (e j) w -> p b e j w", e=2)
        psp8 = psp[:, :, 46:110].rearrange("p b (i j) -> p b i j", j=8)
        psp8q = pspq[:, :, 46:110].rearrange("p b (i j) -> p b i j", j=8)
        psp6q = pspq[:, :, 10:46].rearrange("p b (i j) -> p b i j", j=6)
        psp3q = pspq[:, :, 1:10].rearrange("p b (i j) -> p b i j", j=3)
        psp6 = psp[:, :, 10:46].rearrange("p b (i j) -> p b i j", j=6)
        psp3 = psp[:, :, 1:10].rearrange("p b (i j) -> p b i j", j=3)
        t36_e = t36.rearrange("p b i (j e) -> p b i j e", e=2)
        h8_q = H8.rearrange("p b h (q e f) -> p b h q e f", q=8, e=2, f=4)
        w1_e = W1.rearrange("p b h q (e f) -> p b h q e f", e=2, f=2)

        def finishing2(blo, bhi, qk):
            """s6/s3/s1/s8 finish for a 2-block slice, symmetric-window batched."""
            b = slice(blo, bhi)
            if qk:
                # stash fp8 raw rows as bf16 (TT wants matching input dtypes)
                nc.vector.tensor_copy(rawq[:, 0], pin_r[:, b, :, 10, :])
                nc.vector.tensor_copy(rawq[:, 1], pin_r[:, b, :, 21, :])
                raw03 = rawq[:, 0]
                raw25 = rawq[:, 1]
                p8, p6, p3 = psp8q, psp6q, psp3q
                s1dst = pspq[:, b, 0:1]
            else:
                bb = slice(blo - 2, bhi - 2)
                raw03 = vin_r[:, bb, :, 10, :]
                raw25 = vin_r[:, bb, :, 21, :]
                p8, p6, p3 = psp8, psp6, psp3
                s1dst = psp[:, b, 0:1]
            # s6 h-windows {0,3}: H8{0,4} | H2{4,20} | raw {h10,h42}
            d03 = H6v[:, b, :, 0, :]
            nc.vector.tensor_max(d03, H8v[:, b, :, 0, :], H2v[:, b, :, 4, :])
            nc.vector.tensor_max(d03, d03, raw03)
            # s6 h-windows {1,4}: H2{5,21} | H4{3,11} | H4{4,12} | H2{10,26}
            d14 = H6v[:, b, :, 1, :]
            nc.vector.tensor_max(d14, H2v[:, b, :, 5, :], H4v[:, b, :, 3, :])
            nc.vector.tensor_max(d14, d14, H4v[:, b, :, 4, :])
            nc.vector.tensor_max(d14, d14, H2v[:, b, :, 10, :])
            # s6 h-windows {2,5}: raw {h21,h53} | H2{11,27} | H8{3,7}
            d25 = H6v[:, b, :, 2, :]
            nc.vector.tensor_max(d25, H2v[:, b, :, 11, :], H8v[:, b, :, 3, :])
            nc.vector.tensor_max(d25, d25, raw25)
            # s8 w-tree on H8
            nc.vector.tensor_max(
                W1[:, b], h8_q[:, b, :, :, 0, :], h8_q[:, b, :, :, 1, :]
            )
            nc.vector.tensor_max(
                W2s[:, b], w1_e[:, b, :, :, 0, :], w1_e[:, b, :, :, 1, :]
            )
            nc.vector.tensor_max(
                p8[:, b], W2s[:, b, :, :, 0], W2s[:, b, :, :, 1]
            )
            # s6 w-windows via reduce over [ws,we)
            for j, (ws, we) in enumerate(
                [(0, 11), (10, 22), (21, 32), (32, 43), (42, 54), (53, 64)]
            ):
                nc.vector.reduce_max(
                    p6[:, b, :, j], H6[:, b, :, ws:we], axis=AX.X
                )
            # s3 = 2x2 max over s6 grid
            s6i = p6[:, b].rearrange("p b (i e) j -> p b i e j", e=2)
            nc.vector.tensor_max(t36[:, b], s6i[:, :, :, 0, :], s6i[:, :, :, 1, :])
            nc.vector.tensor_max(
                p3[:, b], t36_e[:, b, :, :, 0], t36_e[:, b, :, :, 1]
            )
            # s1 = max over s8 cells
            nc.vector.reduce_max(
                s1dst.rearrange("p b one -> p (b one)"),
                p8[:, b],
                axis=AX.XY,
            )

        # per-wave psum groups: columns [0,1536) [1536,3072) [3072,4096)
        GDEF = [(0, 3), (3, 6), (6, 8)]

        def conv_group(wt, mofs, dst, m, tag, g, c0, c1):
            vt = psum.tile([128, 1536], f32, tag="psA", bufs=2,
                           name=f"{tag}{m}{g}")
            for kp in range(2):
                for j in range(c1 - c0):
                    nc.tensor.matmul(
                        vt[:, ts(j, COL)],
                        wt[:, 2 * kp : 2 * kp + 2, ts(mofs, 128)],
                        x8s[:, 2 * kp : 2 * kp + 2, ts(c0 + j, COL)],
                        start=(kp == 0),
                        stop=(kp == 1),
                        perf_mode=PM,
                        skip_group_check=True,
                    )
            ncols = (c1 - c0) * COL
            bias = bqk[:, mofs : mofs + 1] if tag == "q" else bvx[:, m : m + 1]
            nc.scalar.activation(
                dst[:, m, c0 * COL : c0 * COL + ncols],
                vt[:, 0:ncols], AF.Relu,
                bias=bias, scale=WUNSCALE,
            )

        def conv_wave(wt, mofs, dst, m, tag):
            """one output-channel block: 3 psum groups, DR matmuls, ACT drain."""
            for g, (c0, c1) in enumerate(GDEF):
                vt = psum.tile([128, 1536], f32, tag="psA", bufs=2,
                               name=f"{tag}{m}{g}")
                for kp in range(2):
                    for j in range(c1 - c0):
                        nc.tensor.matmul(
                            vt[:, ts(j, COL)],
                            wt[:, 2 * kp : 2 * kp + 2, ts(mofs, 128)],
                            x8s[:, 2 * kp : 2 * kp + 2, ts(c0 + j, COL)],
                            start=(kp == 0),
                            stop=(kp == 1),
                            perf_mode=PM,
                            skip_group_check=True,
                        )
                ncols = (c1 - c0) * COL
                bias = bqk[:, mofs : mofs + 1] if tag == "q" else bvx[:, m : m + 1]
                nc.scalar.activation(
                    dst[:, m, c0 * COL : c0 * COL + ncols],
                    vt[:, 0:ncols], AF.Relu,
                    bias=bias, scale=WUNSCALE,
                )

        def block_htree(src_e, bsrc, blk):
            bs = slice(blk, blk + 1)
            bm = slice(bsrc, bsrc + 1)
            nc.vector.tensor_max(
                H2[:, bs], src_e[:, bm, :, 0, :], src_e[:, bm, :, 1, :]
            )
            nc.vector.tensor_max(
                H4[:, bs], h2_e[:, bs, :, 0, :], h2_e[:, bs, :, 1, :]
            )
            nc.vector.tensor_max(
                H8[:, bs], h4_e[:, bs, :, 0, :], h4_e[:, bs, :, 1, :]
            )

        # ---- phase 1: qk conv, 2 m-waves of fp8 DR ----
        for m in range(2):
            conv_wave(wqk, m, pin, m, "q")
            block_htree(pin_e, m, m)

        # ---- phase 2: qk pooling finish (runs during val conv wave 0) ----
        finishing2(0, 2, qk=True)

        # ---- phase 3: val conv m-waves, pooled from PSUM; softmax after w0 ----
        def softmax_chunks(cl):
            for c in cl:
                cs = ts(c, COL)
                ps_s = psum.tile([128, COL], f32, tag="psB", bufs=2, name=f"s{c}")
                nc.tensor.matmul(
                    ps_s[:],
                    pspq[:, 0:2, :],
                    pin[:, 0:2, cs],
                    start=True,
                    stop=True,
                    perf_mode=PM,
                    skip_group_check=True,
                )
                nc.scalar.activation(esc[:, cs], ps_s[0:S, :], AF.Exp, scale=0.0625)
                ps_r = psum.tile([128, COL], f32, tag="psB", bufs=2, name=f"r{c}")
                nc.tensor.matmul(ps_r[:], ones[:], esc[:, cs], start=True, stop=True)
                rf = rpool.tile([128, COL], f32, tag="rf")
                nc.vector.reciprocal_approx_fast(rf[:], ps_r[:])
                rb = rpool.tile([128, COL], bf16, tag="rb")
                nc.scalar.activation(rb[:], rf[:], AF.Identity)
                nc.vector.tensor_mul(esc[:, cs], esc[:, cs], rb[0:S, :])

        SMC = [range(0, 3), range(3, 6), range(6, 8), range(0, 0)]

        def fold_k(ks, start, stop):
            for k in ks:
                nc.tensor.matmul(
                    ps_w[0][:],
                    psp[:, 2 + k, :],
                    w2r[:, k, :],
                    start=start and k == ks[0],
                    stop=stop and k == ks[-1],
                    skip_group_check=True,
                )

        ps_w = [None]
        for m in range(4):
            if m == 3:
                # last wave: tree piece per drained group to shorten the tail
                for g, (c0, c1) in enumerate(GDEF):
                    conv_group(wv, m, vin, m, "v", g, c0, c1)
                    hp0, hp1 = 4 * c0, 4 * c1
                    nc.vector.tensor_max(
                        H2[:, 5, hp0:hp1, :],
                        vin_e[:, m, hp0:hp1, 0, :],
                        vin_e[:, m, hp0:hp1, 1, :],
                    )
                bs = slice(5, 6)
                nc.vector.tensor_max(
                    H4[:, bs], h2_e[:, bs, :, 0, :], h2_e[:, bs, :, 1, :]
                )
                nc.vector.tensor_max(
                    H8[:, bs], h4_e[:, bs, :, 0, :], h4_e[:, bs, :, 1, :]
                )
            else:
                conv_wave(wv, m, vin, m, "v")
                block_htree(vin_e, m, 2 + m)
            softmax_chunks(SMC[m])
            if m == 1:
                finishing2(2, 4, qk=False)
            if m == 2:
                ps_w[0] = psum.tile([S, CIN], f32, tag="psB", bufs=2, name="ps_w")
                fold_k([0, 1], True, False)
                finishing2(4, 5, qk=False)
                fold_k([2], False, False)
            if m == 3:
                finishing2(5, 6, qk=False)

        # ---- phase 4: z waves; fold W2T during wave 0 residuals ----
        ot_prev = None
        for m in range(4):
            zt1 = psum.tile([128, 1536], f32, tag="psA", bufs=2, name=f"za{m}")
            zt2 = psum.tile([128, 1536], f32, tag="psA", bufs=2, name=f"zb{m}")
            for c in range(6):
                nc.tensor.matmul(
                    (zt1 if c < 3 else zt2)[:, ts(c % 3, COL)],
                    ident[:],
                    x32[:, m, ts(c, COL)],
                    start=True,
                    stop=False,
                    skip_group_check=True,
                )
            if m == 0:
                fold_k([3], False, True)
                nc.vector.tensor_copy(w2t[:], ps_w[0][:])
            zt3 = [
                psum.tile([128, COL], f32, tag="psB", bufs=2, name=f"zc{m}{j}")
                for j in range(2)
            ]
            for j in range(2):
                nc.tensor.matmul(
                    zt3[j][:],
                    ident[:],
                    x32[:, m, ts(6 + j, COL)],
                    start=True,
                    stop=False,
                    skip_group_check=True,
                )
            for c in range(NT):
                dst = (
                    zt1[:, ts(c, COL)] if c < 3
                    else zt2[:, ts(c - 3, COL)] if c < 6
                    else zt3[c - 6][:]
                )
                nc.tensor.matmul(
                    dst,
                    w2t[:, ts(m, 128)],
                    esc[:, ts(c, COL)],
                    start=False,
                    stop=True,
                    skip_group_check=True,
                )
            ot = opool.tile([128, NPIX], bf16, tag="ot", name=f"ot{m}")
            nc.scalar.activation(
                ot[:, 0:1536], zt1[:], AF.Identity,
                bias=bvx[:, 8 + m : 9 + m], scale=1.0,
            )
            nc.vector.tensor_scalar(
                ot[:, 1536:3072], zt2[:], bvx[:, 8 + m : 9 + m], None, ALU.add
            )
            nc.scalar.activation(
                ot[:, 3072:3584], zt3[0][:], AF.Identity,
                bias=bvx[:, 8 + m : 9 + m], scale=1.0,
            )
            nc.vector.tensor_scalar(
                ot[:, 3584:4096], zt3[1][:], bvx[:, 8 + m : 9 + m], None, ALU.add
            )
            if m < 3:
                nc.sync.dma_start(out_d[m][:, 0:1536], ot[:, 0:1536])
                nc.sync.dma_start(out_d[m][:, 1536:3072], ot[:, 1536:3072])
                nc.gpsimd.dma_start(out_d[m][:, 3072:4096], ot[:, 3072:4096])
            else:
                nc.sync.dma_start(out_d[m][:, 0:768], ot[:, 0:768])
                nc.gpsimd.dma_start(out_d[m][:, 768:1536], ot[:, 768:1536])
                nc.scalar.dma_start(out_d[m][:, 1536:2304], ot[:, 1536:2304])
                nc.sync.dma_start(out_d[m][:, 2304:3072], ot[:, 2304:3072])
                nc.gpsimd.dma_start(out_d[m][:, 3072:3584], ot[:, 3072:3584])
                nc.scalar.dma_start(out_d[m][:, 3584:4096], ot[:, 3584:4096])

    nc.compile()
    return nc


def _prep_inputs(inputs):
    def f32a(v):
        return np.asarray(v, dtype=np.float32)

    x = f32a(inputs["x"])
    B = x.shape[0]
    qk_w = f32a(inputs["qk_w"])
    v_w = f32a(inputs["v_w"])
    out_w = f32a(inputs["out_w"])

    def fold(w, gamma, beta, mean, var):
        scale = f32a(gamma) / np.sqrt(f32a(var) + EPS)
        return w * scale[:, None], f32a(beta) - f32a(mean) * scale

    wqk, bqk = fold(qk_w, inputs["qk_gamma"], inputs["qk_beta"], inputs["qk_mean"], inputs["qk_var"])
    wv, bv = fold(v_w, inputs["v_gamma"], inputs["v_beta"], inputs["v_mean"], inputs["v_var"])
    wout, bout = fold(out_w, inputs["out_gamma"], inputs["out_beta"], inputs["out_mean"], inputs["out_var"])

    bf = ml_dtypes.bfloat16
    f8 = ml_dtypes.float8_e4m3

    def to_f8(a):
        return np.clip(a, -240.0, 240.0).astype(f8)

    bvx = np.concatenate(
        [
            bv.reshape(4, 128).T,
            (-bv).reshape(4, 128).T,
            bout.reshape(4, 128).T,
        ],
        axis=1,
    ).astype(np.float32)

    shared = {
        "qk_wt": to_f8(np.ascontiguousarray(wqk.T.reshape(4, 128, CK)) * WSCALE),
        "v_wt": to_f8(np.ascontiguousarray(wv.T.reshape(4, 128, CV)) * WSCALE),
        "w2_rhs": np.ascontiguousarray(wout.T.reshape(4, 128, CIN)).astype(bf),
        "b_qk": np.ascontiguousarray(bqk.reshape(2, 128).T.astype(np.float32)),
        "b_vx": np.ascontiguousarray(bvx),
        "ident": np.eye(128, dtype=np.float32).astype(bf),
        "ones": np.ones((S, 128), dtype=np.float32).astype(bf),
    }
    in_maps = []
    for i in range(B):
        m = dict(shared)
        xi = np.ascontiguousarray(x[i].reshape(4, 128, NPIX))
        m["x"] = xi.astype(bf)
        m["x8"] = to_f8(xi)
        in_maps.append(m)
    return in_maps, x.shape


def _run(inputs, trace=False, trace_kwargs=None):
    from concourse.bass_utils import run_bass_kernel_spmd

    if "nc" not in _CACHE:
        _CACHE["nc"] = _build()
    nc = _CACHE["nc"]
    in_maps, xshape = _prep_inputs(inputs)
    res = run_bass_kernel_spmd(
        nc,
        in_maps,
        core_ids=list(range(len(in_maps))),
        trace=trace,
        **(trace_kwargs or {}),
    )
    B = xshape[0]
    out = np.stack(
        [np.asarray(res.results[i]["out"]).astype(np.float32).reshape(CIN, 64, 64) for i in range(B)]
    )
    return out, res


def kernel(**inputs) -> np.ndarray:
    out, _ = _run(inputs, trace=False)
    return out


# revision 40
# speedup vs baseline: 1.0522x; 1.0522x over previous
"""Trainium2 Bass kernel for AsymmetricPositionAttentionModule.

Strategy: pure data parallelism — batch B=8 split across 8 NeuronCores, one
image per core. Per-core graph (convs fp8 DoubleRow, attention bf16, fp32 acc):

  qk  = relu(Wqk8'·x8 + bqk)     fp8 DR matmuls -> ACT relu drain (bf16 pin)
  val = Wv8'·x8                  fp8 DR matmuls -> pooled DIRECTLY from PSUM
                                 (relu+bias applied post-pooling on psp: max
                                  commutes with the monotone relu(x+b))
  key/valPSP = PSP maxpool       DVE/Pool max-tree, symmetric-window batching
  scoresT = keyT·qk              PE, [110, 512] per chunk
  esc = exp(scores/16)           ACT, bf16
  sums = onesT·esc               PE broadcast trick
  escn = esc * recip(sums)       DVE recip, ACT cast, DVE mult
  W2T = valPSP·Wout'T            PE [110, 512] (folds out-conv with value)
  z = W2T.T·escn + I·xb          PE, residual via identity matmul (bf16 x)
  out = z + bout                 ACT/DVE epilogue, bf16 out
"""

import sys

sys.path.insert(0, "/opt/trn_rl_repo")

from contextlib import ExitStack

import numpy as np
import ml_dtypes

CIN = 512
CK = 256
CV = 512
NPIX = 4096
S = 110
NT = 8          # pixel columns of 512
COL = 512
EPS = 1e-5
WSCALE = 1024.0   # pow2 boost for folded conv weights into fp8 range
WUNSCALE = 1.0 / WSCALE

_CACHE = {}


def _build():
    import concourse.bass as bass
    import concourse.tile as tile
    from concourse import bacc, mybir

    f32 = mybir.dt.float32
    bf16 = mybir.dt.bfloat16
    fp8 = mybir.dt.float8e4
    ts = bass.ts
    AF = mybir.ActivationFunctionType
    ALU = mybir.AluOpType
    AX = mybir.AxisListType
    PM = mybir.MatmulPerfMode.DoubleRow

    nc = bacc.Bacc("TRN2", target_bir_lowering=False, debug=False, num_devices=8)

    x_d = nc.dram_tensor("x", [4, 128, NPIX], bf16, kind="ExternalInput").ap()
    x8_d = nc.dram_tensor("x8", [4, 128, NPIX], fp8, kind="ExternalInput").ap()
    qkw_d = nc.dram_tensor("qk_wt", [4, 128, CK], fp8, kind="ExternalInput").ap()
    vw_d = nc.dram_tensor("v_wt", [4, 128, CV], fp8, kind="ExternalInput").ap()
    w2_d = nc.dram_tensor("w2_rhs", [4, 128, CIN], bf16, kind="ExternalInput").ap()
    bqk_d = nc.dram_tensor("b_qk", [128, 2], f32, kind="ExternalInput").ap()
    bvx_d = nc.dram_tensor("b_vx", [128, 12], f32, kind="ExternalInput").ap()
    id_d = nc.dram_tensor("ident", [128, 128], bf16, kind="ExternalInput").ap()
    ones_d = nc.dram_tensor("ones", [S, 128], bf16, kind="ExternalInput").ap()
    out_d = nc.dram_tensor("out", [4, 128, NPIX], bf16, kind="ExternalOutput").ap()

    with tile.TileContext(nc) as tc, ExitStack() as ctx:
        const = ctx.enter_context(tc.tile_pool(name="const", bufs=1))
        persist = ctx.enter_context(tc.tile_pool(name="persist", bufs=1))
        rpool = ctx.enter_context(tc.tile_pool(name="rpool", bufs=2))
        opool = ctx.enter_context(tc.tile_pool(name="opool", bufs=2))
        psum = ctx.enter_context(tc.tile_pool(name="psum", bufs=2, space="PSUM"))

        # ---- constants ----
        wqk = const.tile([128, 4, CK], fp8)
        wv = const.tile([128, 4, CV], fp8)
        w2r = const.tile([128, 4, CIN], bf16)
        bqk = const.tile([128, 2], f32)
        bvx = const.tile([128, 12], f32)   # bv 0:4 | -bv 4:8 | bout 8:12
        ident = const.tile([128, 128], bf16)
        ones = const.tile([S, 128], bf16)
        # ---- persistent activations ----
        x32 = persist.tile([128, 4, NPIX], bf16)      # input (bf16), residual only
        x8s = persist.tile([128, 4, NPIX], fp8)       # input (fp8), conv operand
        pin = persist.tile([128, 2, NPIX], fp8)       # qk activations (fp8)
        pspq = persist.tile([128, 2, 128], fp8)       # pooled key, fp8, padded
        rawq = persist.tile([128, 2, 2, 2, 64], bf16) # qk raw rows (d03|d25)
        vin = persist.tile([128, 4, NPIX], bf16)      # val activations
        H2 = persist.tile([128, 6, 32, 64], bf16)
        H4 = persist.tile([128, 6, 16, 64], bf16)
        H8 = persist.tile([128, 6, 8, 64], bf16)
        H6 = persist.tile([128, 6, 6, 64], bf16)
        t36 = persist.tile([128, 6, 3, 6], bf16)
        W1 = persist.tile([128, 6, 8, 8, 4], bf16)
        W2s = persist.tile([128, 6, 8, 8, 2], bf16)
        psp = persist.tile([128, 6, S], bf16)         # pooled: [s1|s3|s6|s8]
        esc = persist.tile([S, NPIX], bf16)
        w2t = persist.tile([S, CIN], bf16)

        # x8 lands group-by-group across 3 queues; a single dma_start only
        # sustains ~40GB/s, so split into per-(k, column-group) pieces.
        Q3 = [nc.sync, nc.scalar, nc.gpsimd]
        for k in range(4):          # group 0 in half-pieces, round-robin
            for h, (a, b) in enumerate([(0, 768), (768, 1536)]):
                Q3[(2 * k + h) % 3].dma_start(x8s[:, k, a:b], x8_d[k][:, a:b])
        for k in range(4):
            nc.scalar.dma_start(wqk[:, k, :], qkw_d[k])
        nc.scalar.dma_start(bqk[:], bqk_d)
        for gi, (a, b) in enumerate([(1536, 3072), (3072, 4096)]):
            nc.sync.dma_start(x8s[:, 0, a:b], x8_d[0][:, a:b])
            nc.scalar.dma_start(x8s[:, 1, a:b], x8_d[1][:, a:b])
            nc.sync.dma_start(x8s[:, 2, a:b], x8_d[2][:, a:b])
            nc.gpsimd.dma_start(x8s[:, 3, a:b], x8_d[3][:, a:b])
        for k in range(4):
            nc.gpsimd.dma_start(wv[:, k, :], vw_d[k])
            nc.gpsimd.dma_start(w2r[:, k, :], w2_d[k])
        nc.gpsimd.dma_start(bvx[:], bvx_d)
        nc.gpsimd.dma_start(ident[:], id_d)
        nc.gpsimd.dma_start(ones[:], ones_d)
        for k in range(4):
            nc.gpsimd.dma_start(x32[:, k, :], x_d[k])

        nc.vector.memset(pspq[:, :, S:128], 0)

        # views of pin for the pooling tree (free dim = h*64 + w)
        pin_e = pin.rearrange("p b (hp e w) -> p b hp e w", e=2, w=64)
        pin_r = pin.rearrange("p b (e h w) -> p b e h w", e=2, w=64)  # h half-split
        vin_e = vin.rearrange("p b (hp e w) -> p b hp e w", e=2, w=64)
        vin_r = vin.rearrange("p b (e h w) -> p b e h w", e=2, w=64)
        h2_e = H2.rearrange("p b (hp e) w -> p b hp e w", e=2)
        h4_e = H4.rearrange("p b (hp e) w -> p b hp e w", e=2)
        H2v = H2.rearrange("p b (e h) w -> p b e h w", e=2)
        H4v = H4.rearrange("p b (e h) w -> p b e h w", e=2)
        H8v = H8.rearrange("p b (e h) w -> p b e h w", e=2)
        H6v = H6.rearrange("p b (e j) w -> p b e j w", e=2)
        psp8 = psp[:, :, 46:110].rearrange("p b (i j) -> p b i j", j=8)
        psp8q = pspq[:, :, 46:110].rearrange("p b (i j) -> p b i j", j=8)
        psp6q = pspq[:, :, 10:46].rearrange("p b (i j) -> p b i j", j=6)
        psp3q = pspq[:, :, 1:10].rearrange("p b (i j) -> p b i j", j=3)
        psp6 = psp[:, :, 10:46].rearrange("p b (i j) -> p b i j", j=6)
        psp3 = psp[:, :, 1:10].rearrange("p b (i j) -> p b i j", j=3)
        t36_e = t36.rearrange("p b i (j e) -> p b i j e", e=2)
        h8_q = H8.rearrange("p b h (q e f) -> p b h q e f", q=8, e=2, f=4)
        w1_e = W1.rearrange("p b h q (e f) -> p b h q e f", e=2, f=2)

        def finishing2(blo, bhi, qk):
            """s6/s3/s1/s8 finish for a 2-block slice, symmetric-window batched."""
            b = slice(blo, bhi)
            if qk:
                # stash fp8 raw rows as bf16 (TT wants matching input dtypes)
                nc.vector.tensor_copy(rawq[:, 0], pin_r[:, b, :, 10, :])
                nc.vector.tensor_copy(rawq[:, 1], pin_r[:, b, :, 21, :])
                raw03 = rawq[:, 0]
                raw25 = rawq[:, 1]
                p8, p6, p3 = psp8q, psp6q, psp3q
                s1dst = pspq[:, b, 0:1]
            else:
                bb = slice(blo - 2, bhi - 2)
                raw03 = vin_r[:, bb, :, 10, :]
                raw25 = vin_r[:, bb, :, 21, :]
                p8, p6, p3 = psp8, psp6, psp3
                s1dst = psp[:, b, 0:1]
            # s6 h-windows {0,3}: H8{0,4} | H2{4,20} | raw {h10,h42}
            d03 = H6v[:, b, :, 0, :]
            nc.vector.tensor_max(d03, H8v[:, b, :, 0, :], H2v[:, b, :, 4, :])
            nc.vector.tensor_max(d03, d03, raw03)
            # s6 h-windows {1,4}: H2{5,21} | H4{3,11} | H4{4,12} | H2{10,26}
            d14 = H6v[:, b, :, 1, :]
            nc.vector.tensor_max(d14, H2v[:, b, :, 5, :], H4v[:, b, :, 3, :])
            nc.vector.tensor_max(d14, d14, H4v[:, b, :, 4, :])
            nc.vector.tensor_max(d14, d14, H2v[:, b, :, 10, :])
            # s6 h-windows {2,5}: raw {h21,h53} | H2{11,27} | H8{3,7}
            d25 = H6v[:, b, :, 2, :]
            nc.vector.tensor_max(d25, H2v[:, b, :, 11, :], H8v[:, b, :, 3, :])
            nc.vector.tensor_max(d25, d25, raw25)
            # s8 w-tree on H8
            nc.vector.tensor_max(
                W1[:, b], h8_q[:, b, :, :, 0, :], h8_q[:, b, :, :, 1, :]
            )
            nc.vector.tensor_max(
                W2s[:, b], w1_e[:, b, :, :, 0, :], w1_e[:, b, :, :, 1, :]
            )
            nc.vector.tensor_max(
                p8[:, b], W2s[:, b, :, :, 0], W2s[:, b, :, :, 1]
            )
            # s6 w-windows via reduce over [ws,we)
            for j, (ws, we) in enumerate(
                [(0, 11), (10, 22), (21, 32), (32, 43), (42, 54), (53, 64)]
            ):
                nc.vector.reduce_max(
                    p6[:, b, :, j], H6[:, b, :, ws:we], axis=AX.X
                )
            # s3 = 2x2 max over s6 grid
            s6i = p6[:, b].rearrange("p b (i e) j -> p b i e j", e=2)
            nc.vector.tensor_max(t36[:, b], s6i[:, :, :, 0, :], s6i[:, :, :, 1, :])
            nc.vector.tensor_max(
                p3[:, b], t36_e[:, b, :, :, 0], t36_e[:, b, :, :, 1]
            )
            # s1 = max over s8 cells
            nc.vector.reduce_max(
                s1dst.rearrange("p b one -> p (b one)"),
                p8[:, b],
                axis=AX.XY,
            )

        # per-wave psum groups: columns [0,1536) [1536,3072) [3072,4096)
        GDEF = [(0, 2), (2, 4), (4, 6), (6, 8)]

        def conv_group(wt, mofs, dst, m, tag, g, c0, c1):
            vt = psum.tile([128, 1024], f32, tag="psA", bufs=3,
                           name=f"{tag}{m}{g}")
            for kp in range(2):
                for j in range(c1 - c0):
                    nc.tensor.matmul(
                        vt[:, ts(j, COL)],
                        wt[:, 2 * kp : 2 * kp + 2, ts(mofs, 128)],
                        x8s[:, 2 * kp : 2 * kp + 2, ts(c0 + j, COL)],
                        start=(kp == 0),
                        stop=(kp == 1),
                        perf_mode=PM,
                        skip_group_check=True,
                    )
            ncols = (c1 - c0) * COL
            bias = bqk[:, mofs : mofs + 1] if tag == "q" else bvx[:, m : m + 1]
            nc.scalar.activation(
                dst[:, m, c0 * COL : c0 * COL + ncols],
                vt[:, 0:ncols], AF.Relu,
                bias=bias, scale=WUNSCALE,
            )

        def conv_wave(wt, mofs, dst, m, tag):
            """one output-channel block: 3 psum groups, DR matmuls, ACT drain."""
            for g, (c0, c1) in enumerate(GDEF):
                vt = psum.tile([128, 1024], f32, tag="psA", bufs=3,
                               name=f"{tag}{m}{g}")
                for kp in range(2):
                    for j in range(c1 - c0):
                        nc.tensor.matmul(
                            vt[:, ts(j, COL)],
                            wt[:, 2 * kp : 2 * kp + 2, ts(mofs, 128)],
                            x8s[:, 2 * kp : 2 * kp + 2, ts(c0 + j, COL)],
                            start=(kp == 0),
                            stop=(kp == 1),
                            perf_mode=PM,
                            skip_group_check=True,
                        )
                ncols = (c1 - c0) * COL
                bias = bqk[:, mofs : mofs + 1] if tag == "q" else bvx[:, m : m + 1]
                nc.scalar.activation(
                    dst[:, m, c0 * COL : c0 * COL + ncols],
                    vt[:, 0:ncols], AF.Relu,
                    bias=bias, scale=WUNSCALE,
                )

        def block_htree(src_e, bsrc, blk):
            bs = slice(blk, blk + 1)
            bm = slice(bsrc, bsrc + 1)
            nc.vector.tensor_max(
                H2[:, bs], src_e[:, bm, :, 0, :], src_e[:, bm, :, 1, :]
            )
            nc.vector.tensor_max(
                H4[:, bs], h2_e[:, bs, :, 0, :], h2_e[:, bs, :, 1, :]
            )
            nc.vector.tensor_max(
                H8[:, bs], h4_e[:, bs, :, 0, :], h4_e[:, bs, :, 1, :]
            )

        # ---- phase 1: qk conv, 2 m-waves of fp8 DR ----
        for m in range(2):
            conv_wave(wqk, m, pin, m, "q")
            block_htree(pin_e, m, m)

        # ---- phase 2: qk pooling finish (runs during val conv wave 0) ----
        finishing2(0, 2, qk=True)

        # ---- phase 3: val conv m-waves, pooled from PSUM; softmax after w0 ----
        def softmax_chunks(cl):
            for c in cl:
                cs = ts(c, COL)
                ps_s = psum.tile([128, COL], f32, tag="psB", bufs=2, name=f"s{c}")
                nc.tensor.matmul(
                    ps_s[:],
                    pspq[:, 0:2, :],
                    pin[:, 0:2, cs],
                    start=True,
                    stop=True,
                    perf_mode=PM,
                    skip_group_check=True,
                )
                nc.scalar.activation(esc[:, cs], ps_s[0:S, :], AF.Exp, scale=0.0625)
                ps_r = psum.tile([128, COL], f32, tag="psB", bufs=2, name=f"r{c}")
                nc.tensor.matmul(ps_r[:], ones[:], esc[:, cs], start=True, stop=True)
                rf = rpool.tile([128, COL], f32, tag="rf")
                nc.vector.reciprocal_approx_fast(rf[:], ps_r[:])
                rb = rpool.tile([128, COL], bf16, tag="rb")
                nc.scalar.activation(rb[:], rf[:], AF.Identity)
                nc.vector.tensor_mul(esc[:, cs], esc[:, cs], rb[0:S, :])

        SMC = [range(0, 3), range(3, 6), range(6, 8), range(0, 0)]

        def fold_k(ks, start, stop):
            for k in ks:
                nc.tensor.matmul(
                    ps_w[0][:],
                    psp[:, 2 + k, :],
                    w2r[:, k, :],
                    start=start and k == ks[0],
                    stop=stop and k == ks[-1],
                    skip_group_check=True,
                )

        ps_w = [None]
        for m in range(4):
            if m == 3:
                # last wave: tree piece per drained group to shorten the tail
                for g, (c0, c1) in enumerate(GDEF):
                    conv_group(wv, m, vin, m, "v", g, c0, c1)
                    hp0, hp1 = 4 * c0, 4 * c1
                    nc.vector.tensor_max(
                        H2[:, 5, hp0:hp1, :],
                        vin_e[:, m, hp0:hp1, 0, :],
                        vin_e[:, m, hp0:hp1, 1, :],
                    )
                bs = slice(5, 6)
                nc.vector.tensor_max(
                    H4[:, bs], h2_e[:, bs, :, 0, :], h2_e[:, bs, :, 1, :]
                )
                nc.vector.tensor_max(
                    H8[:, bs], h4_e[:, bs, :, 0, :], h4_e[:, bs, :, 1, :]
                )
            else:
                conv_wave(wv, m, vin, m, "v")
                block_htree(vin_e, m, 2 + m)
            softmax_chunks(SMC[m])
            if m == 1:
                finishing2(2, 4, qk=False)
            if m == 2:
                ps_w[0] = psum.tile([S, CIN], f32, tag="psB", bufs=2, name="ps_w")
                fold_k([0, 1], True, False)
                finishing2(4, 5, qk=False)
                fold_k([2], False, False)
            if m == 3:
                finishing2(5, 6, qk=False)

        # ---- phase 4: z waves; fold W2T during wave 0 residuals ----
        ot_prev = None
        for m in range(4):
            zts = [
                psum.tile([128, 1024], f32, tag="psA", bufs=3, name=f"z{m}{j}")
                for j in range(3)
            ]
            for c in range(6):
                nc.tensor.matmul(
                    zts[c // 2][:, ts(c % 2, COL)],
                    ident[:],
                    x32[:, m, ts(c, COL)],
                    start=True,
                    stop=False,
                    skip_group_check=True,
                )
            if m == 0:
                fold_k([3], False, True)
                nc.vector.tensor_copy(w2t[:], ps_w[0][:])
            zt3 = [
                psum.tile([128, COL], f32, tag="psB", bufs=2, name=f"zc{m}{j}")
                for j in range(2)
            ]
            for j in range(2):
                nc.tensor.matmul(
                    zt3[j][:],
                    ident[:],
                    x32[:, m, ts(6 + j, COL)],
                    start=True,
                    stop=False,
                    skip_group_check=True,
                )
            for c in range(NT):
                dst = zts[c // 2][:, ts(c % 2, COL)] if c < 6 else zt3[c - 6][:]
                nc.tensor.matmul(
                    dst,
                    w2t[:, ts(m, 128)],
                    esc[:, ts(c, COL)],
                    start=False,
                    stop=True,
                    skip_group_check=True,
                )
            ot = opool.tile([128, NPIX], bf16, tag="ot", name=f"ot{m}")
            nc.scalar.activation(
                ot[:, 0:1024], zts[0][:], AF.Identity,
                bias=bvx[:, 8 + m : 9 + m], scale=1.0,
            )
            nc.vector.tensor_scalar(
                ot[:, 1024:2048], zts[1][:], bvx[:, 8 + m : 9 + m], None, ALU.add
            )
            nc.scalar.activation(
                ot[:, 2048:3072], zts[2][:], AF.Identity,
                bias=bvx[:, 8 + m : 9 + m], scale=1.0,
            )
            nc.vector.tensor_scalar(
                ot[:, 3072:3584], zt3[0][:], bvx[:, 8 + m : 9 + m], None, ALU.add
            )
            nc.scalar.activation(
                ot[:, 3584:4096], zt3[1][:], AF.Identity,
                bias=bvx[:, 8 + m : 9 + m], scale=1.0,
            )
            if m < 3:
                nc.sync.dma_start(out_d[m][:, 0:1536], ot[:, 0:1536])
                nc.sync.dma_start(out_d[m][:, 1536:3072], ot[:, 1536:3072])
                nc.gpsimd.dma_start(out_d[m][:, 3072:4096], ot[:, 3072:4096])
            else:
                nc.sync.dma_start(out_d[m][:, 0:768], ot[:, 0:768])
                nc.gpsimd.dma_start(out_d[m][:, 768:1536], ot[:, 768:1536])
                nc.scalar.dma_start(out_d[m][:, 1536:2304], ot[:, 1536:2304])
                nc.sync.dma_start(out_d[m][:, 2304:3072], ot[:, 2304:3072])
                nc.gpsimd.dma_start(out_d[m][:, 3072:3584], ot[:, 3072:3584])
                nc.scalar.dma_start(out_d[m][:, 3584:4096], ot[:, 3584:4096])

    nc.compile()
    return nc


def _prep_inputs(inputs):
    def f32a(v):
        return np.asarray(v, dtype=np.float32)

    x = f32a(inputs["x"])
    B = x.shape[0]
    qk_w = f32a(inputs["qk_w"])
    v_w = f32a(inputs["v_w"])
    out_w = f32a(inputs["out_w"])

    def fold(w, gamma, beta, mean, var):
        scale = f32a(gamma) / np.sqrt(f32a(var) + EPS)
        return w * scale[:, None], f32a(beta) - f32a(mean) * scale

    wqk, bqk = fold(qk_w, inputs["qk_gamma"], inputs["qk_beta"], inputs["qk_mean"], inputs["qk_var"])
    wv, bv = fold(v_w, inputs["v_gamma"], inputs["v_beta"], inputs["v_mean"], inputs["v_var"])
    wout, bout = fold(out_w, inputs["out_gamma"], inputs["out_beta"], inputs["out_mean"], inputs["out_var"])

    bf = ml_dtypes.bfloat16
    f8 = ml_dtypes.float8_e4m3

    def to_f8(a):
        return np.clip(a, -240.0, 240.0).astype(f8)

    bvx = np.concatenate(
        [
            bv.reshape(4, 128).T,
            (-bv).reshape(4, 128).T,
            bout.reshape(4, 128).T,
        ],
        axis=1,
    ).astype(np.float32)

    shared = {
        "qk_wt": to_f8(np.ascontiguousarray(wqk.T.reshape(4, 128, CK)) * WSCALE),
        "v_wt": to_f8(np.ascontiguousarray(wv.T.reshape(4, 128, CV)) * WSCALE),
        "w2_rhs": np.ascontiguousarray(wout.T.reshape(4, 128, CIN)).astype(bf),
        "b_qk": np.ascontiguousarray(bqk.reshape(2, 128).T.astype(np.float32)),
        "b_vx": np.ascontiguousarray(bvx),
        "ident": np.eye(128, dtype=np.float32).astype(bf),
        "ones": np.ones((S, 128), dtype=np.float32).astype(bf),
    }
    in_maps = []
    for i in range(B):
        m = dict(shared)
        xi = np.ascontiguousarray(x[i].reshape(4, 128, NPIX))
        m["x"] = xi.astype(bf)
        m["x8"] = to_f8(xi)
        in_maps.append(m)
    return in_maps, x.shape


def _run(inputs, trace=False, trace_kwargs=None):
    from concourse.bass_utils import run_bass_kernel_spmd

    if "nc" not in _CACHE:
        _CACHE["nc"] = _build()
    nc = _CACHE["nc"]
    in_maps, xshape = _prep_inputs(inputs)
    res = run_bass_kernel_spmd(
        nc,
        in_maps,
        core_ids=list(range(len(in_maps))),
        trace=trace,
        **(trace_kwargs or {}),
    )
    B = xshape[0]
    out = np.stack(
        [np.asarray(res.results[i]["out"]).astype(np.float32).reshape(CIN, 64, 64) for i in range(B)]
    )
    return out, res


def kernel(**inputs) -> np.ndarray:
    out, _ = _run(inputs, trace=False)
    return out


# revision 41
# speedup vs baseline: 1.0589x; 1.0064x over previous
"""Trainium2 Bass kernel for AsymmetricPositionAttentionModule.

Strategy: pure data parallelism — batch B=8 split across 8 NeuronCores, one
image per core. Per-core graph (convs fp8 DoubleRow, attention bf16, fp32 acc):

  qk  = relu(Wqk8'·x8 + bqk)     fp8 DR matmuls -> ACT relu drain (bf16 pin)
  val = Wv8'·x8                  fp8 DR matmuls -> pooled DIRECTLY from PSUM
                                 (relu+bias applied post-pooling on psp: max
                                  commutes with the monotone relu(x+b))
  key/valPSP = PSP maxpool       DVE/Pool max-tree, symmetric-window batching
  scoresT = keyT·qk              PE, [110, 512] per chunk
  esc = exp(scores/16)           ACT, bf16
  sums = onesT·esc               PE broadcast trick
  escn = esc * recip(sums)       DVE recip, ACT cast, DVE mult
  W2T = valPSP·Wout'T            PE [110, 512] (folds out-conv with value)
  z = W2T.T·escn + I·xb          PE, residual via identity matmul (bf16 x)
  out = z + bout                 ACT/DVE epilogue, bf16 out
"""

import sys

sys.path.insert(0, "/opt/trn_rl_repo")

from contextlib import ExitStack

import numpy as np
import ml_dtypes

CIN = 512
CK = 256
CV = 512
NPIX = 4096
S = 110
NT = 8          # pixel columns of 512
COL = 512
EPS = 1e-5
WSCALE = 1024.0   # pow2 boost for folded conv weights into fp8 range
WUNSCALE = 1.0 / WSCALE

_CACHE = {}


def _build():
    import concourse.bass as bass
    import concourse.tile as tile
    from concourse import bacc, mybir

    f32 = mybir.dt.float32
    bf16 = mybir.dt.bfloat16
    fp8 = mybir.dt.float8e4
    ts = bass.ts
    AF = mybir.ActivationFunctionType
    ALU = mybir.AluOpType
    AX = mybir.AxisListType
    PM = mybir.MatmulPerfMode.DoubleRow

    nc = bacc.Bacc("TRN2", target_bir_lowering=False, debug=False, num_devices=8)

    x_d = nc.dram_tensor("x", [4, 128, NPIX], bf16, kind="ExternalInput").ap()
    x8_d = nc.dram_tensor("x8", [4, 128, NPIX], fp8, kind="ExternalInput").ap()
    qkw_d = nc.dram_tensor("qk_wt", [4, 128, CK], fp8, kind="ExternalInput").ap()
    vw_d = nc.dram_tensor("v_wt", [4, 128, CV], fp8, kind="ExternalInput").ap()
    w2_d = nc.dram_tensor("w2_rhs", [4, 128, CIN], bf16, kind="ExternalInput").ap()
    bqk_d = nc.dram_tensor("b_qk", [128, 2], f32, kind="ExternalInput").ap()
    bvx_d = nc.dram_tensor("b_vx", [128, 12], f32, kind="ExternalInput").ap()
    id_d = nc.dram_tensor("ident", [128, 128], bf16, kind="ExternalInput").ap()
    ones_d = nc.dram_tensor("ones", [S, 128], bf16, kind="ExternalInput").ap()
    out_d = nc.dram_tensor("out", [4, 128, NPIX], bf16, kind="ExternalOutput").ap()

    with tile.TileContext(nc) as tc, ExitStack() as ctx:
        const = ctx.enter_context(tc.tile_pool(name="const", bufs=1))
        persist = ctx.enter_context(tc.tile_pool(name="persist", bufs=1))
        rpool = ctx.enter_context(tc.tile_pool(name="rpool", bufs=2))
        opool = ctx.enter_context(tc.tile_pool(name="opool", bufs=2))
        psum = ctx.enter_context(tc.tile_pool(name="psum", bufs=2, space="PSUM"))

        # ---- constants ----
        wqk = const.tile([128, 4, CK], fp8)
        wv = const.tile([128, 4, CV], fp8)
        w2r = const.tile([128, 4, CIN], bf16)
        bqk = const.tile([128, 2], f32)
        bvx = const.tile([128, 12], f32)   # bv 0:4 | -bv 4:8 | bout 8:12
        ident = const.tile([128, 128], bf16)
        ones = const.tile([S, 128], bf16)
        # ---- persistent activations ----
        x32 = persist.tile([128, 4, NPIX], bf16)      # input (bf16), residual only
        x8s = persist.tile([128, 4, NPIX], fp8)       # input (fp8), conv operand
        pin = persist.tile([128, 2, NPIX], fp8)       # qk activations (fp8)
        pspq = persist.tile([128, 2, 128], fp8)       # pooled key, fp8, padded
        rawq = persist.tile([128, 2, 2, 2, 64], bf16) # qk raw rows (d03|d25)
        vin = persist.tile([128, 4, NPIX], bf16)      # val activations
        H2 = persist.tile([128, 6, 32, 64], bf16)
        H4 = persist.tile([128, 6, 16, 64], bf16)
        H8 = persist.tile([128, 6, 8, 64], bf16)
        H6 = persist.tile([128, 6, 6, 64], bf16)
        t36 = persist.tile([128, 6, 3, 6], bf16)
        W1 = persist.tile([128, 6, 8, 8, 4], bf16)
        W2s = persist.tile([128, 6, 8, 8, 2], bf16)
        psp = persist.tile([128, 6, S], bf16)         # pooled: [s1|s3|s6|s8]
        esc = persist.tile([S, NPIX], bf16)
        w2t = persist.tile([S, CIN], bf16)

        # x8 lands group-by-group across 3 queues; a single dma_start only
        # sustains ~40GB/s, so split into per-(k, column-group) pieces.
        Q3 = [nc.sync, nc.scalar, nc.gpsimd]
        for k in range(4):          # group 0 in half-pieces, round-robin
            for h, (a, b) in enumerate([(0, 768), (768, 1536)]):
                Q3[(2 * k + h) % 3].dma_start(x8s[:, k, a:b], x8_d[k][:, a:b])
        for k in range(4):
            nc.scalar.dma_start(wqk[:, k, :], qkw_d[k])
        nc.scalar.dma_start(bqk[:], bqk_d)
        for gi, (a, b) in enumerate([(1536, 3072), (3072, 4096)]):
            nc.sync.dma_start(x8s[:, 0, a:b], x8_d[0][:, a:b])
            nc.scalar.dma_start(x8s[:, 1, a:b], x8_d[1][:, a:b])
            nc.sync.dma_start(x8s[:, 2, a:b], x8_d[2][:, a:b])
            nc.gpsimd.dma_start(x8s[:, 3, a:b], x8_d[3][:, a:b])
        for k in range(4):
            nc.sync.dma_start(wv[:, k, :], vw_d[k])
            nc.sync.dma_start(w2r[:, k, :], w2_d[k])
        nc.sync.dma_start(bvx[:], bvx_d)
        nc.sync.dma_start(ident[:], id_d)
        nc.sync.dma_start(ones[:], ones_d)
        for k in range(4):
            nc.sync.dma_start(x32[:, k, :], x_d[k])

        nc.vector.memset(pspq[:, :, S:128], 0)

        # views of pin for the pooling tree (free dim = h*64 + w)
        pin_e = pin.rearrange("p b (hp e w) -> p b hp e w", e=2, w=64)
        pin_r = pin.rearrange("p b (e h w) -> p b e h w", e=2, w=64)  # h half-split
        vin_e = vin.rearrange("p b (hp e w) -> p b hp e w", e=2, w=64)
        vin_r = vin.rearrange("p b (e h w) -> p b e h w", e=2, w=64)
        h2_e = H2.rearrange("p b (hp e) w -> p b hp e w", e=2)
        h4_e = H4.rearrange("p b (hp e) w -> p b hp e w", e=2)
        H2v = H2.rearrange("p b (e h) w -> p b e h w", e=2)
        H4v = H4.rearrange("p b (e h) w -> p b e h w", e=2)
        H8v = H8.rearrange("p b (e h) w -> p b e h w", e=2)
        H6v = H6.rearrange("p b (e j) w -> p b e j w", e=2)
        psp8 = psp[:, :, 46:110].rearrange("p b (i j) -> p b i j", j=8)
        psp8q = pspq[:, :, 46:110].rearrange("p b (i j) -> p b i j", j=8)
        psp6q = pspq[:, :, 10:46].rearrange("p b (i j) -> p b i j", j=6)
        psp3q = pspq[:, :, 1:10].rearrange("p b (i j) -> p b i j", j=3)
        psp6 = psp[:, :, 10:46].rearrange("p b (i j) -> p b i j", j=6)
        psp3 = psp[:, :, 1:10].rearrange("p b (i j) -> p b i j", j=3)
        t36_e = t36.rearrange("p b i (j e) -> p b i j e", e=2)
        h8_q = H8.rearrange("p b h (q e f) -> p b h q e f", q=8, e=2, f=4)
        w1_e = W1.rearrange("p b h q (e f) -> p b h q e f", e=2, f=2)

        def finishing2(blo, bhi, qk):
            """s6/s3/s1/s8 finish for a 2-block slice, symmetric-window batched."""
            b = slice(blo, bhi)
            if qk:
                # stash fp8 raw rows as bf16 (TT wants matching input dtypes)
                nc.vector.tensor_copy(rawq[:, 0], pin_r[:, b, :, 10, :])
                nc.vector.tensor_copy(rawq[:, 1], pin_r[:, b, :, 21, :])
                raw03 = rawq[:, 0]
                raw25 = rawq[:, 1]
                p8, p6, p3 = psp8q, psp6q, psp3q
                s1dst = pspq[:, b, 0:1]
            else:
                bb = slice(blo - 2, bhi - 2)
                raw03 = vin_r[:, bb, :, 10, :]
                raw25 = vin_r[:, bb, :, 21, :]
                p8, p6, p3 = psp8, psp6, psp3
                s1dst = psp[:, b, 0:1]
            # s6 h-windows {0,3}: H8{0,4} | H2{4,20} | raw {h10,h42}
            d03 = H6v[:, b, :, 0, :]
            nc.vector.tensor_max(d03, H8v[:, b, :, 0, :], H2v[:, b, :, 4, :])
            nc.vector.tensor_max(d03, d03, raw03)
            # s6 h-windows {1,4}: H2{5,21} | H4{3,11} | H4{4,12} | H2{10,26}
            d14 = H6v[:, b, :, 1, :]
            nc.vector.tensor_max(d14, H2v[:, b, :, 5, :], H4v[:, b, :, 3, :])
            nc.vector.tensor_max(d14, d14, H4v[:, b, :, 4, :])
            nc.vector.tensor_max(d14, d14, H2v[:, b, :, 10, :])
            # s6 h-windows {2,5}: raw {h21,h53} | H2{11,27} | H8{3,7}
            d25 = H6v[:, b, :, 2, :]
            nc.vector.tensor_max(d25, H2v[:, b, :, 11, :], H8v[:, b, :, 3, :])
            nc.vector.tensor_max(d25, d25, raw25)
            # s8 w-tree on H8
            nc.vector.tensor_max(
                W1[:, b], h8_q[:, b, :, :, 0, :], h8_q[:, b, :, :, 1, :]
            )
            nc.vector.tensor_max(
                W2s[:, b], w1_e[:, b, :, :, 0, :], w1_e[:, b, :, :, 1, :]
            )
            nc.vector.tensor_max(
                p8[:, b], W2s[:, b, :, :, 0], W2s[:, b, :, :, 1]
            )
            # s6 w-windows via reduce over [ws,we)
            for j, (ws, we) in enumerate(
                [(0, 11), (10, 22), (21, 32), (32, 43), (42, 54), (53, 64)]
            ):
                nc.vector.reduce_max(
                    p6[:, b, :, j], H6[:, b, :, ws:we], axis=AX.X
                )
            # s3 = 2x2 max over s6 grid
            s6i = p6[:, b].rearrange("p b (i e) j -> p b i e j", e=2)
            nc.vector.tensor_max(t36[:, b], s6i[:, :, :, 0, :], s6i[:, :, :, 1, :])
            nc.vector.tensor_max(
                p3[:, b], t36_e[:, b, :, :, 0], t36_e[:, b, :, :, 1]
            )
            # s1 = max over s8 cells
            nc.vector.reduce_max(
                s1dst.rearrange("p b one -> p (b one)"),
                p8[:, b],
                axis=AX.XY,
            )

        # per-wave psum groups: columns [0,1536) [1536,3072) [3072,4096)
        GDEF = [(0, 2), (2, 4), (4, 6), (6, 8)]

        def conv_group(wt, mofs, dst, m, tag, g, c0, c1):
            vt = psum.tile([128, 1024], f32, tag="psA", bufs=3,
                           name=f"{tag}{m}{g}")
            for kp in range(2):
                for j in range(c1 - c0):
                    nc.tensor.matmul(
                        vt[:, ts(j, COL)],
                        wt[:, 2 * kp : 2 * kp + 2, ts(mofs, 128)],
                        x8s[:, 2 * kp : 2 * kp + 2, ts(c0 + j, COL)],
                        start=(kp == 0),
                        stop=(kp == 1),
                        perf_mode=PM,
                        skip_group_check=True,
                    )
            ncols = (c1 - c0) * COL
            bias = bqk[:, mofs : mofs + 1] if tag == "q" else bvx[:, m : m + 1]
            nc.scalar.activation(
                dst[:, m, c0 * COL : c0 * COL + ncols],
                vt[:, 0:ncols], AF.Relu,
                bias=bias, scale=WUNSCALE,
            )

        def conv_wave(wt, mofs, dst, m, tag):
            """one output-channel block: 3 psum groups, DR matmuls, ACT drain."""
            for g, (c0, c1) in enumerate(GDEF):
                vt = psum.tile([128, 1024], f32, tag="psA", bufs=3,
                               name=f"{tag}{m}{g}")
                for kp in range(2):
                    for j in range(c1 - c0):
                        nc.tensor.matmul(
                            vt[:, ts(j, COL)],
                            wt[:, 2 * kp : 2 * kp + 2, ts(mofs, 128)],
                            x8s[:, 2 * kp : 2 * kp + 2, ts(c0 + j, COL)],
                            start=(kp == 0),
                            stop=(kp == 1),
                            perf_mode=PM,
                            skip_group_check=True,
                        )
                ncols = (c1 - c0) * COL
                bias = bqk[:, mofs : mofs + 1] if tag == "q" else bvx[:, m : m + 1]
                nc.scalar.activation(
                    dst[:, m, c0 * COL : c0 * COL + ncols],
                    vt[:, 0:ncols], AF.Relu,
                    bias=bias, scale=WUNSCALE,
                )

        def block_htree(src_e, bsrc, blk):
            bs = slice(blk, blk + 1)
            bm = slice(bsrc, bsrc + 1)
            nc.vector.tensor_max(
                H2[:, bs], src_e[:, bm, :, 0, :], src_e[:, bm, :, 1, :]
            )
            nc.vector.tensor_max(
                H4[:, bs], h2_e[:, bs, :, 0, :], h2_e[:, bs, :, 1, :]
            )
            nc.vector.tensor_max(
                H8[:, bs], h4_e[:, bs, :, 0, :], h4_e[:, bs, :, 1, :]
            )

        # ---- phase 1: qk conv, 2 m-waves of fp8 DR ----
        for m in range(2):
            conv_wave(wqk, m, pin, m, "q")
            block_htree(pin_e, m, m)

        # ---- phase 2: qk pooling finish (runs during val conv wave 0) ----
        finishing2(0, 2, qk=True)

        # ---- phase 3: val conv m-waves, pooled from PSUM; softmax after w0 ----
        def softmax_chunks(cl):
            for c in cl:
                cs = ts(c, COL)
                ps_s = psum.tile([128, COL], f32, tag="psB", bufs=2, name=f"s{c}")
                nc.tensor.matmul(
                    ps_s[:],
                    pspq[:, 0:2, :],
                    pin[:, 0:2, cs],
                    start=True,
                    stop=True,
                    perf_mode=PM,
                    skip_group_check=True,
                )
                nc.scalar.activation(esc[:, cs], ps_s[0:S, :], AF.Exp, scale=0.0625)
                ps_r = psum.tile([128, COL], f32, tag="psB", bufs=2, name=f"r{c}")
                nc.tensor.matmul(ps_r[:], ones[:], esc[:, cs], start=True, stop=True)
                rf = rpool.tile([128, COL], f32, tag="rf")
                nc.vector.reciprocal_approx_fast(rf[:], ps_r[:])
                rb = rpool.tile([128, COL], bf16, tag="rb")
                nc.scalar.activation(rb[:], rf[:], AF.Identity)
                nc.vector.tensor_mul(esc[:, cs], esc[:, cs], rb[0:S, :])

        SMC = [range(0, 3), range(3, 6), range(6, 8), range(0, 0)]

        def fold_k(ks, start, stop):
            for k in ks:
                nc.tensor.matmul(
                    ps_w[0][:],
                    psp[:, 2 + k, :],
                    w2r[:, k, :],
                    start=start and k == ks[0],
                    stop=stop and k == ks[-1],
                    skip_group_check=True,
                )

        ps_w = [None]
        for m in range(4):
            if m == 3:
                # last wave: tree piece per drained group to shorten the tail
                for g, (c0, c1) in enumerate(GDEF):
                    conv_group(wv, m, vin, m, "v", g, c0, c1)
                    hp0, hp1 = 4 * c0, 4 * c1
                    nc.vector.tensor_max(
                        H2[:, 5, hp0:hp1, :],
                        vin_e[:, m, hp0:hp1, 0, :],
                        vin_e[:, m, hp0:hp1, 1, :],
                    )
                bs = slice(5, 6)
                nc.vector.tensor_max(
                    H4[:, bs], h2_e[:, bs, :, 0, :], h2_e[:, bs, :, 1, :]
                )
                nc.vector.tensor_max(
                    H8[:, bs], h4_e[:, bs, :, 0, :], h4_e[:, bs, :, 1, :]
                )
            else:
                conv_wave(wv, m, vin, m, "v")
                block_htree(vin_e, m, 2 + m)
            softmax_chunks(SMC[m])
            if m == 1:
                finishing2(2, 4, qk=False)
            if m == 2:
                ps_w[0] = psum.tile([S, CIN], f32, tag="psB", bufs=2, name="ps_w")
                fold_k([0, 1], True, False)
                finishing2(4, 5, qk=False)
                fold_k([2], False, False)
            if m == 3:
                finishing2(5, 6, qk=False)

        # ---- phase 4: z waves; fold W2T during wave 0 residuals ----
        ot_prev = None
        for m in range(4):
            zts = [
                psum.tile([128, 1024], f32, tag="psA", bufs=3, name=f"z{m}{j}")
                for j in range(3)
            ]
            for c in range(6):
                nc.tensor.matmul(
                    zts[c // 2][:, ts(c % 2, COL)],
                    ident[:],
                    x32[:, m, ts(c, COL)],
                    start=True,
                    stop=False,
                    skip_group_check=True,
                )
            if m == 0:
                fold_k([3], False, True)
                nc.vector.tensor_copy(w2t[:], ps_w[0][:])
            zt3 = [
                psum.tile([128, COL], f32, tag="psB", bufs=2, name=f"zc{m}{j}")
                for j in range(2)
            ]
            for j in range(2):
                nc.tensor.matmul(
                    zt3[j][:],
                    ident[:],
                    x32[:, m, ts(6 + j, COL)],
                    start=True,
                    stop=False,
                    skip_group_check=True,
                )
            for c in range(NT):
                dst = zts[c // 2][:, ts(c % 2, COL)] if c < 6 else zt3[c - 6][:]
                nc.tensor.matmul(
                    dst,
                    w2t[:, ts(m, 128)],
                    esc[:, ts(c, COL)],
                    start=False,
                    stop=True,
                    skip_group_check=True,
                )
            ot = opool.tile([128, NPIX], bf16, tag="ot", name=f"ot{m}")
            nc.scalar.activation(
                ot[:, 0:1024], zts[0][:], AF.Identity,
                bias=bvx[:, 8 + m : 9 + m], scale=1.0,
            )
            nc.vector.tensor_scalar(
                ot[:, 1024:2048], zts[1][:], bvx[:, 8 + m : 9 + m], None, ALU.add
            )
            nc.scalar.activation(
                ot[:, 2048:3072], zts[2][:], AF.Identity,
                bias=bvx[:, 8 + m : 9 + m], scale=1.0,
            )
            nc.vector.tensor_scalar(
                ot[:, 3072:3584], zt3[0][:], bvx[:, 8 + m : 9 + m], None, ALU.add
            )
            nc.scalar.activation(
                ot[:, 3584:4096], zt3[1][:], AF.Identity,
                bias=bvx[:, 8 + m : 9 + m], scale=1.0,
            )
            if m < 3:
                nc.sync.dma_start(out_d[m][:, 0:1536], ot[:, 0:1536])
                nc.sync.dma_start(out_d[m][:, 1536:3072], ot[:, 1536:3072])
                nc.gpsimd.dma_start(out_d[m][:, 3072:4096], ot[:, 3072:4096])
            else:
                nc.sync.dma_start(out_d[m][:, 0:768], ot[:, 0:768])
                nc.gpsimd.dma_start(out_d[m][:, 768:1536], ot[:, 768:1536])
                nc.scalar.dma_start(out_d[m][:, 1536:2304], ot[:, 1536:2304])
                nc.sync.dma_start(out_d[m][:, 2304:3072], ot[:, 2304:3072])
                nc.gpsimd.dma_start(out_d[m][:, 3072:3584], ot[:, 3072:3584])
                nc.scalar.dma_start(out_d[m][:, 3584:4096], ot[:, 3584:4096])

    nc.compile()
    return nc


def _prep_inputs(inputs):
    def f32a(v):
        return np.asarray(v, dtype=np.float32)

    x = f32a(inputs["x"])
    B = x.shape[0]
    qk_w = f32a(inputs["qk_w"])
    v_w = f32a(inputs["v_w"])
    out_w = f32a(inputs["out_w"])

    def fold(w, gamma, beta, mean, var):
        scale = f32a(gamma) / np.sqrt(f32a(var) + EPS)
        return w * scale[:, None], f32a(beta) - f32a(mean) * scale

    wqk, bqk = fold(qk_w, inputs["qk_gamma"], inputs["qk_beta"], inputs["qk_mean"], inputs["qk_var"])
    wv, bv = fold(v_w, inputs["v_gamma"], inputs["v_beta"], inputs["v_mean"], inputs["v_var"])
    wout, bout = fold(out_w, inputs["out_gamma"], inputs["out_beta"], inputs["out_mean"], inputs["out_var"])

    bf = ml_dtypes.bfloat16
    f8 = ml_dtypes.float8_e4m3

    def to_f8(a):
        return np.clip(a, -240.0, 240.0).astype(f8)

    bvx = np.concatenate(
        [
            bv.reshape(4, 128).T,
            (-bv).reshape(4, 128).T,
            bout.reshape(4, 128).T,
        ],
        axis=1,
    ).astype(np.float32)

    shared = {
        "qk_wt": to_f8(np.ascontiguousarray(wqk.T.reshape(4, 128, CK)) * WSCALE),
        "v_wt": to_f8(np.ascontiguousarray(wv.T.reshape(4, 128, CV)) * WSCALE),
        "w2_rhs": np.ascontiguousarray(wout.T.reshape(4, 128, CIN)).astype(bf),
        "b_qk": np.ascontiguousarray(bqk.reshape(2, 128).T.astype(np.float32)),
        "b_vx": np.ascontiguousarray(bvx),
        "ident": np.eye(128, dtype=np.float32).astype(bf),
        "ones": np.ones((S, 128), dtype=np.float32).astype(bf),
    }
    in_maps = []
    for i in range(B):
        m = dict(shared)
        xi = np.ascontiguousarray(x[i].reshape(4, 128, NPIX))
        m["x"] = xi.astype(bf)
        m["x8"] = to_f8(xi)
        in_maps.append(m)
    return in_maps, x.shape


def _run(inputs, trace=False, trace_kwargs=None):
    from concourse.bass_utils import run_bass_kernel_spmd

    if "nc" not in _CACHE:
        _CACHE["nc"] = _build()
    nc = _CACHE["nc"]
    in_maps, xshape = _prep_inputs(inputs)
    res = run_bass_kernel_spmd(
        nc,
        in_maps,
        core_ids=list(range(len(in_maps))),
        trace=trace,
        **(trace_kwargs or {}),
    )
    B = xshape[0]
    out = np.stack(
        [np.asarray(res.results[i]["out"]).astype(np.float32).reshape(CIN, 64, 64) for i in range(B)]
    )
    return out, res


def kernel(**inputs) -> np.ndarray:
    out, _ = _run(inputs, trace=False)
    return out
